# revision 20
# baseline (speedup 1.0000x reference)
"""Trainium2 Bass kernel for causal GQA self-attention with partial RoPE,
per-head qk gains, and xsa (self-component subtraction along v).

Sharding: 8 cores = (B=2 batches) x (NKV=4 kv heads). Each core handles one
batch and one kv group (4 query heads + 1 kv head), computes its partial
output projection, and the host sums the 4 partials per batch (the
"all-reduce after c_proj" done on host during unshard).

All layout transposes are done host-side in numpy during input sharding, so
the device kernel runs zero PE transposes except the small v-chunk transposes
needed for the PV matmul stationary operand.
"""

import sys
import numpy as np

if "/opt/trn_rl_repo" not in sys.path:
    sys.path.insert(0, "/opt/trn_rl_repo")

B, T, D = 2, 2048, 1024
NH, NKV, HD = 16, 4, 64
ROPE, HALF = 16, 8
GROUPS = NH // NKV          # 4 query heads per kv head
NCORE = 8
NJ, JW = 2, 1024            # query superblocks
NC, CW = 16, 128            # key chunks

_cache = {}


def _bank_segments(lo, end=JW):
    """Column segments [s0, s1) covering [lo, end), each inside a single
    512-float PSUM bank (matmul outputs cannot cross bank boundaries)."""
    segs = []
    for seg in range(0, end, 512):
        s0, s1 = max(lo, seg), min(end, seg + 512)
        if s0 < s1:
            segs.append((s0, s1 - s0))
    return segs


def _build_nc():
    import concourse.bacc as bacc
    import concourse.mybir as mybir
    import concourse.tile as tile

    dt = mybir.dt
    f32 = dt.float32
    f32r = dt.float32r
    Act = mybir.ActivationFunctionType

    nc = bacc.Bacc("TRN2", target_bir_lowering=False, debug=False,
                   num_devices=NCORE)

    # DRAM I/O (per-core shards prepared on host)
    xT_d = nc.dram_tensor("xT", [D, T], f32, kind="ExternalInput")
    wq_d = nc.dram_tensor("wq", [D, GROUPS * HD], f32, kind="ExternalInput")
    wkv_d = nc.dram_tensor("wkv", [D, 2 * HD], f32, kind="ExternalInput")
    wo_d = nc.dram_tensor("wo", [GROUPS * HD, D], f32, kind="ExternalInput")
    Cm_d = nc.dram_tensor("Cm", [128, T], f32, kind="ExternalInput")
    Sm_d = nc.dram_tensor("Sm", [128, T], f32, kind="ExternalInput")
    dm_d = nc.dram_tensor("dm", [128, 128], f32, kind="ExternalInput")
    id_d = nc.dram_tensor("id64", [64, 64], f32, kind="ExternalInput")
    on_d = nc.dram_tensor("on64", [64, 1], f32, kind="ExternalInput")
    outp_d = nc.dram_tensor("outp", [T, D], f32, kind="ExternalOutput")

    V, S, PE, GP, SY = nc.vector, nc.scalar, nc.tensor, nc.gpsimd, nc.sync

    with tile.TileContext(nc) as tc:
        with tc.tile_pool(name="sb", bufs=1) as sb, \
             tc.tile_pool(name="ps", bufs=1, space="PSUM") as ps:

            # ---- constants ----
            Cs = sb.tile([64, T], f32, name="Cs", tag="scr", bufs=4)
            SY.dma_start(Cs[:], Cm_d.ap()[0:64, :])
            Ss = sb.tile([64, T], f32, name="Ss", tag="scr", bufs=4)
            SY.dma_start(Ss[:], Sm_d.ap()[0:64, :])
            dm = sb.tile([128, 128], f32, name="dm")
            SY.dma_start(dm[:], dm_d.ap())
            idt = sb.tile([64, 64], f32, name="idt")
            SY.dma_start(idt[:], id_d.ap())
            on1 = sb.tile([64, 1], f32, name="on1")
            SY.dma_start(on1[:], on_d.ap())

            # ---- weights ----
            wqs = []
            for d in range(8):
                wqt = sb.tile([128, GROUPS * HD], f32, name=f"wq{d}",
                              tag="wq", bufs=8)
                SY.dma_start(wqt[:], wq_d.ap()[d * 128:(d + 1) * 128, :])
                wqs.append(wqt)
            wkvs = []
            for d in range(8):
                wkt = sb.tile([128, 2 * HD], f32, name=f"wkv{d}",
                              tag="wkv", bufs=8)
                SY.dma_start(wkt[:], wkv_d.ap()[d * 128:(d + 1) * 128, :])
                wkvs.append(wkt)
            wos = []
            for f in range(2):
                wot = sb.tile([128, D], f32, name=f"wo{f}", tag="wo", bufs=2)
                SY.dma_start(wot[:], wo_d.ap()[f * 128:(f + 1) * 128, :])
                wos.append(wot)

            # ---- destination tiles for qkv (transposed feature-major) ----
            # separate base-0 tiles per head: matmul requires lhsT and rhs to
            # share the same base partition.
            qTh = [sb.tile([64, T], f32, name=f"qT{i}", tag="qT", bufs=4)
                   for i in range(4)]
            kT = sb.tile([64, T], f32, name="kT")
            vT = sb.tile([64, T], f32, name="vT")

            # ---- phase 1: qkv projection (feature-major output) ----
            for t in range(4):
                xts = []
                for d in range(8):
                    xt = sb.tile([128, 512], f32, name=f"x{t}_{d}",
                                 tag="xt", bufs=8)
                    SY.dma_start(
                        xt[:],
                        xT_d.ap()[d * 128:(d + 1) * 128, t * 512:(t + 1) * 512])
                    xts.append(xt)
                for m in range(3):
                    p = ps.tile([128, 512], f32, name=f"qkvP{t}_{m}",
                                tag="big", bufs=3)
                    for d in range(8):
                        if m < 2:
                            lhs = wqs[d][:, m * 128:(m + 1) * 128]
                        else:
                            lhs = wkvs[d][:]
                        PE.matmul(p[:], lhs.bitcast(f32r),
                                  xts[d][:].bitcast(f32r),
                                  start=(d == 0), stop=(d == 7))
                    if m < 2:
                        V.tensor_copy(qTh[2 * m][:, t * 512:(t + 1) * 512],
                                      p[0:64, :])
                        V.tensor_copy(qTh[2 * m + 1][:, t * 512:(t + 1) * 512],
                                      p[64:128, :])
                    else:
                        V.tensor_copy(kT[:, t * 512:(t + 1) * 512], p[0:64, :])
                        V.tensor_copy(vT[:, t * 512:(t + 1) * 512],
                                      p[64:128, :])

            # ---- phase 2: partial rope on q heads and k ----
            # qsw holds the 8-row-swapped copy of the rope rows; rows >= 16
            # are multiplied by S=0 so only rope rows matter, but memset once
            # so stale SBUF bits can't produce NaN * 0.
            for tgt_i, tgt in enumerate(qTh + [kT]):
                qsw = sb.tile([64, T], f32, name=f"qsw{tgt_i}",
                              tag="scr", bufs=4)
                GP.memset(qsw[:], 0.0)
                SY.dma_start(qsw[0:8, :], tgt[8:16, :])
                SY.dma_start(qsw[8:16, :], tgt[0:8, :])
                t1 = sb.tile([64, T], f32, name=f"rs1_{tgt_i}",
                             tag="scr", bufs=4)
                V.tensor_mul(t1[:], tgt[:], Cs[:])
                V.tensor_mul(qsw[:], qsw[:], Ss[:])
                V.tensor_add(tgt[:], t1[:], qsw[:])

            # ---- phase 2b: build [v | 1] key-major tiles for PV ----
            vk1_all = sb.tile([128, NC * 65], f32, name="vk1_all")
            vk1 = [vk1_all[:, c * 65:(c + 1) * 65] for c in range(NC)]
            for c in range(NC):
                vp = ps.tile([128, 64], f32, name=f"vtP{c}", tag="sm", bufs=2)
                PE.transpose(vp[:], vT[:, c * 128:(c + 1) * 128], idt[:])
                V.tensor_copy(vk1[c][:, 0:64], vp[:])
                GP.memset(vk1[c][:, 64:65], 1.0)

            # ---- stat tiles: rows at legal partition starts {0,32,64,96} ----
            statA = sb.tile([128, T], f32, name="statA")  # r0: vnorm, r32: rn
            statB = sb.tile([128, T], f32, name="statB")  # r32h: dots/coef h
            statC = sb.tile([128, T], f32, name="statC")  # r32h: denom/recip h

            # ---- phase 2c: vnorm = sum_d v^2, rn = 1/max(vnorm, eps) ----
            vsq = sb.tile([64, T], f32, name="vsq", tag="scr", bufs=4)
            V.tensor_mul(vsq[:], vT[:], vT[:])
            for ch in range(4):
                stp = ps.tile([1, 512], f32, name=f"nrm{ch}", tag="sm", bufs=2)
                PE.matmul(stp[:], on1[:].bitcast(f32r),
                          vsq[:, ch * 512:(ch + 1) * 512].bitcast(f32r),
                          start=True, stop=True)
                V.tensor_copy(statA[0:1, ch * 512:(ch + 1) * 512], stp[:])
            V.tensor_scalar_max(statA[32:33, :], statA[0:1, :], 1e-8)
            V.reciprocal(statA[32:33, :], statA[32:33, :])

            # ---- phase 3: attention per head ----
            aoF = [sb.tile([128, T], f32, name=f"aoF{i}", tag="aoF", bufs=2)
                   for i in range(2)]
            Exp = Act.Exp
            for h in range(GROUPS):
                qsrc = qTh[h]
                aoU = sb.tile([64, T], f32, name=f"aoU{h}", tag="aoU", bufs=1)
                for J in range(NJ):
                    cmax = 8 * J + 7
                    acc = ps.tile([65, JW], f32, name=f"acc{h}_{J}",
                                  tag="big", bufs=3)
                    for c in range(cmax + 1):
                        lo = max(0, 128 * c - JW * J)
                        span = JW - lo
                        sp = ps.tile([128, JW], f32, name=f"sc{h}_{J}_{c}",
                                     tag="big", bufs=3)
                        for s0, w in _bank_segments(lo):
                            PE.matmul(
                                sp[:, s0:s0 + w],
                                kT[:, c * 128:(c + 1) * 128].bitcast(f32r),
                                qsrc[:, JW * J + s0:
                                     JW * J + s0 + w].bitcast(f32r),
                                start=True, stop=True)
                        if c >= 8 * J:
                            V.tensor_add(sp[:, lo:lo + 128],
                                         sp[:, lo:lo + 128], dm[:])
                        pb = sb.tile([128, JW], f32, name=f"pb{h}_{J}_{c}",
                                     tag="probs", bufs=2)
                        S.activation(pb[:, 0:span], sp[:, lo:lo + span], Exp)
                        for s0, w in _bank_segments(lo):
                            PE.matmul(acc[:, s0:s0 + w],
                                      vk1[c][:].bitcast(f32r),
                                      pb[:, s0 - lo:s0 - lo + w].bitcast(f32r),
                                      start=(c == 0), stop=(c == cmax),
                                      skip_group_check=True)
                    V.tensor_copy(aoU[:, JW * J:JW * (J + 1)], acc[0:64, :])
                    V.tensor_copy(statC[32 * h:32 * h + 1, JW * J:JW * (J + 1)],
                                  acc[64:65, :])
                # xsa stats: dots_h = sum_d aoU * vT (unnormalized)
                prod = sb.tile([64, T], f32, name=f"prod{h}",
                               tag="scr", bufs=4)
                V.tensor_mul(prod[:], aoU[:], vT[:])
                for ch in range(4):
                    stp = ps.tile([1, 512], f32, name=f"dot{h}_{ch}",
                                  tag="sm", bufs=2)
                    PE.matmul(stp[:], on1[:].bitcast(f32r),
                              prod[:, ch * 512:(ch + 1) * 512].bitcast(f32r),
                              start=True, stop=True)
                    V.tensor_copy(statB[32 * h:32 * h + 1,
                                        ch * 512:(ch + 1) * 512], stp[:])
                # r_h = 1/denom ; c2_h = dots * rn * r_h (in place)
                V.reciprocal(statC[32 * h:32 * h + 1, :],
                             statC[32 * h:32 * h + 1, :])
                V.tensor_mul(statB[32 * h:32 * h + 1, :],
                             statB[32 * h:32 * h + 1, :], statA[32:33, :])
                V.tensor_mul(statB[32 * h:32 * h + 1, :],
                             statB[32 * h:32 * h + 1, :],
                             statC[32 * h:32 * h + 1, :])
                # broadcast across 64 partitions and combine:
                # aoF_h = aoU * r_b - vT * c2_b
                c2b = sb.tile([64, T], f32, name=f"c2b{h}", tag="bb", bufs=2)
                GP.partition_broadcast(c2b[:], statB[32 * h:32 * h + 1, :])
                rb = sb.tile([64, T], f32, name=f"rb{h}", tag="bb", bufs=2)
                GP.partition_broadcast(rb[:], statC[32 * h:32 * h + 1, :])
                x1 = sb.tile([64, T], f32, name=f"xs1_{h}", tag="scr", bufs=4)
                x2 = sb.tile([64, T], f32, name=f"xs2_{h}", tag="scr", bufs=4)
                V.tensor_mul(x1[:], aoU[:], rb[:])
                V.tensor_mul(x2[:], vT[:], c2b[:])
                V.tensor_sub(aoF[h // 2][(h % 2) * 64:(h % 2) * 64 + 64, :],
                             x1[:], x2[:])

            # ---- phase 4: partial out-projection ----
            for t in range(16):
                pp = ps.tile([128, D], f32, name=f"prj{t}", tag="big", bufs=3)
                for f in range(2):
                    for jh in range(2):
                        PE.matmul(
                            pp[:, jh * 512:(jh + 1) * 512],
                            aoF[f][:, t * 128:(t + 1) * 128].bitcast(f32r),
                            wos[f][:, jh * 512:(jh + 1) * 512].bitcast(f32r),
                            start=(f == 0), stop=(f == 1))
                ob = sb.tile([128, D], f32, name=f"ob{t}", tag="outB", bufs=2)
                V.tensor_copy(ob[:], pp[:])
                SY.dma_start(outp_d.ap()[t * 128:(t + 1) * 128, :], ob[:])

    nc.compile()
    return nc


def _get_nc():
    if "nc" not in _cache:
        _cache["nc"] = _build_nc()
    return _cache["nc"]


def _prep_core_inputs(x, cos, sin, w_qkv, w_out, q_scale, k_scale):
    """Build the 8 per-core input dicts (host-side sharding + transposes)."""
    cosT = np.ascontiguousarray(cos.T, dtype=np.float32)   # [8, T]
    sinT = np.ascontiguousarray(sin.T, dtype=np.float32)
    Cm = np.zeros((128, T), np.float32)
    Sm = np.zeros((128, T), np.float32)
    for kblk in range(2):
        r = 64 * kblk
        Cm[r:r + 8] = cosT
        Cm[r + 8:r + 16] = cosT
        Cm[r + 16:r + 64] = 1.0
        Sm[r:r + 8] = -sinT
        Sm[r + 8:r + 16] = sinT
    ii = np.arange(128)
    dmask = np.where(ii[None, :] >= ii[:, None], 0.0, -1e30).astype(np.float32)
    id64 = np.eye(64, dtype=np.float32)
    on64 = np.ones((64, 1), np.float32)

    in_maps = []
    scale = 1.0 / np.sqrt(HD)
    for core in range(NCORE):
        b, g = core // NKV, core % NKV
        heads = range(GROUPS * g, GROUPS * g + GROUPS)
        wq_rows = np.concatenate(
            [w_qkv[h * HD:(h + 1) * HD] * (q_scale[h, 0] * scale)
             for h in heads], axis=0)                       # [256, D]
        wk_rows = w_qkv[NH * HD + g * HD: NH * HD + (g + 1) * HD] \
            * k_scale[g, 0]
        wv_rows = w_qkv[(NH + NKV) * HD + g * HD:
                        (NH + NKV) * HD + (g + 1) * HD]
        in_maps.append({
            "xT": np.ascontiguousarray(x[b].T, dtype=np.float32),
            "wq": np.ascontiguousarray(wq_rows.T, dtype=np.float32),
            "wkv": np.ascontiguousarray(
                np.concatenate([wk_rows, wv_rows], axis=0).T,
                dtype=np.float32),
            "wo": np.ascontiguousarray(
                w_out[:, g * GROUPS * HD:(g + 1) * GROUPS * HD].T,
                dtype=np.float32),
            "Cm": Cm, "Sm": Sm, "dm": dmask, "id64": id64, "on64": on64,
        })
    return in_maps


def _fallback_numpy(x, cos, sin, attn_mask, w_qkv, w_out, q_scale, k_scale):
    """Reference-equivalent numpy path for non-causal masks."""
    x64 = x.astype(np.float64)
    qkv = x64 @ w_qkv.T.astype(np.float64)
    q = qkv[..., :NH * HD].reshape(B, T, NH, HD).transpose(0, 2, 1, 3)
    k = qkv[..., NH * HD:(NH + NKV) * HD].reshape(B, T, NKV, HD) \
        .transpose(0, 2, 1, 3)
    v = qkv[..., (NH + NKV) * HD:].reshape(B, T, NKV, HD).transpose(0, 2, 1, 3)
    q = q * q_scale[None, :, :, None]
    k = k * k_scale[None, :, :, None]

    def rope(t):
        c = cos[None, None].astype(np.float64)
        s = sin[None, None].astype(np.float64)
        t1, t2, tp = t[..., :HALF], t[..., HALF:ROPE], t[..., ROPE:]
        return np.concatenate([t1 * c - t2 * s, t1 * s + t2 * c, tp], axis=-1)

    q, k = rope(q), rope(k)
    ke = np.repeat(k, GROUPS, axis=1)
    ve = np.repeat(v, GROUPS, axis=1)
    scores = np.einsum("bhqd,bhkd->bhqk", q, ke) / np.sqrt(HD)
    scores = np.where(attn_mask[None, None], scores, -np.inf)
    m = scores.max(axis=-1, keepdims=True)
    m = np.where(np.isfinite(m), m, 0.0)
    e = np.exp(scores - m)
    probs = e / e.sum(axis=-1, keepdims=True)
    ao = np.einsum("bhqk,bhkd->bhqd", probs, ve)
    ao_r = ao.reshape(B, NKV, GROUPS, T, HD)
    vv = v[:, :, None]
    dots = np.sum(ao_r * vv, axis=-1, keepdims=True)
    nrm = np.maximum(np.sum(vv * vv, axis=-1, keepdims=True), 1e-8)
    ao = (ao_r - (dots / nrm) * vv).reshape(B, NH, T, HD)
    out = ao.transpose(0, 2, 1, 3).reshape(B, T, D) @ w_out.T.astype(np.float64)
    return out.astype(np.float32)


def kernel(x, cos, sin, attn_mask, w_qkv, w_out, q_scale, k_scale):
    from concourse import bass_utils

    x = np.asarray(x, np.float32)
    cos = np.asarray(cos, np.float32)
    sin = np.asarray(sin, np.float32)
    w_qkv = np.asarray(w_qkv, np.float32)
    w_out = np.asarray(w_out, np.float32)
    q_scale = np.asarray(q_scale, np.float32)
    k_scale = np.asarray(k_scale, np.float32)
    am = np.asarray(attn_mask, bool)

    tril = np.tril(np.ones((T, T), dtype=bool))
    if am.shape != (T, T) or not np.array_equal(am, tril):
        return _fallback_numpy(x, cos, sin, am, w_qkv, w_out,
                               q_scale, k_scale)

    nc = _get_nc()
    in_maps = _prep_core_inputs(x, cos, sin, w_qkv, w_out, q_scale, k_scale)
    res = bass_utils.run_bass_kernel_spmd(nc, in_maps,
                                          core_ids=list(range(NCORE)))
    out = np.zeros((B, T, D), np.float32)
    for core in range(NCORE):
        out[core // NKV] += res.results[core]["outp"]
    return out
